# revision 62
# baseline (speedup 1.0000x reference)
"""Causal multi-head self-attention on 8 Trainium2 NeuronCores.

Sharding: hybrid (batch x head-half) tensor parallel. B=4 batches x 2
head-groups of 8 heads = 8 shards. Core c = (b = c//2, g = c%2) gets batch
b's X (transposed, loaded once) and the W_qkv rows / W_out columns for heads
8g..8g+7, computes those heads' attention plus its half of the output
projection contraction, and returns a partial [S, D] output. The host sums
the 2 partials per batch (the "all-reduce" of the TP output projection).

vs the pure head-parallel V1: X DMA per core drops 16MB -> 4MB (loaded
once), y output 32MB -> 8MB, and the output projection accumulates its 4
contraction tiles in PSUM so it is evacuated once instead of 4x.

On-device layout (per core; everything transposed until the end):
  - v natural [kpos, e] is computed directly by matmul (lhsT = X^T tile,
    rhs = W_v^T tile) -- no PE transposes -- and scattered into per-head-pair
    blocks [V_h0 | 1 | V_h1 | 1]; the ones columns make the attn@V matmul
    emit softmax denominators in PSUM row 64 for free.
  - qT/kT per head-pair: [128 rows = 2 heads x dv, S]; head0 on partitions
    0-63, head1 on 64-127.
  - scoresT[kpos, q] = matmul(lhsT=kT tile, rhs=qT tile) per head; K=64 with
    lhsT/rhs base partitions 0/64 auto-derives PE row-tiling positions
    (0,0)/(64,0), so the two heads' score matmuls run concurrently on the
    array halves.
  - causal: only q >= kpos tiles/columns computed (partial-width matmuls);
    the diagonal 128-block gets a 0/1 triangular mask multiplied after exp.
  - softmax without max-subtraction (scores ~ N(0,1); fp32 exp is safe);
    exp on the scalar engine reads PSUM directly, one call per pair covering
    both heads.
  - normalization: reciprocal of the PSUM denominator row, partition-
    broadcast via a DRAM bounce, one DVE multiply into outT.
  - emission interleaving: the next head-pair's Q/K projection matmuls are
    emitted between attention pairs ("fillers") so the in-order PE queue has
    work while the scalar engine computes exp; the output projection tiles
    are likewise interleaved into the last head-pair's attention.
"""

import numpy as np

import concourse.bacc as bacc
import concourse.bass as bass
import concourse.mybir as mybir
import concourse.tile as tile

FP32 = mybir.dt.float32

B = 4
S = 2048
D = 1024
H = 16
DV = 64
N_CORES = 8
H_LOC = 8                  # heads per core
N_HP = H_LOC // 2          # head-pairs per core
E_LOC = H_LOC * DV         # 512 rows of Q/K/V per core
MM_DT = mybir.dt.float16

SQ = 512            # q tile width (PSUM bank)
SK = 128            # kpos tile width
N_SQ = S // SQ      # 4 q-tiles
N_SK = S // SK      # 16 kpos tiles
N_D = D // 128      # 8 contraction tiles for QKV
VBLK = 2 * (DV + 1)  # 130: [V_h0 | 1 | V_h1 | 1] per (kpos tile, head-pair)


def build_nc() -> bass.Bass:
    # Bacc (not plain Bass): its compile() pass splits multi-wait
    # instructions that walrus codegen otherwise rejects.
    nc = bacc.Bacc(None, target_bir_lowering=False)

    xt = nc.declare_dram_parameter("xt", [D, S], MM_DT, isOutput=False)
    wqT = nc.declare_dram_parameter("wqT", [D, E_LOC], MM_DT, isOutput=False)
    wkT = nc.declare_dram_parameter("wkT", [D, E_LOC], MM_DT, isOutput=False)
    wvT = nc.declare_dram_parameter("wvT", [D, E_LOC], MM_DT, isOutput=False)
    woutT = nc.declare_dram_parameter("woutT", [E_LOC, D], MM_DT, isOutput=False)
    y = nc.declare_dram_parameter("y", [S, D], FP32, isOutput=True)

    with tile.TileContext(nc) as tc:
        _build(tc, xt, wqT, wkT, wvT, woutT, y)
    nc.compile()
    return nc


def _build(tc, xt, wqT, wkT, wvT, woutT, y):
    nc = tc.nc

    def mm(out, lhsT, rhs, start, stop):
        nc.tensor.matmul(out, lhsT=lhsT, rhs=rhs, start=start, stop=stop)

    with (
        tc.tile_pool(name="consts", bufs=1) as consts,
        tc.tile_pool(name="qkp", bufs=2) as qkp,
        tc.tile_pool(name="attnp", bufs=1) as attnp,
        tc.tile_pool(name="outp", bufs=1) as outp,
        tc.tile_pool(name="outh1p", bufs=2) as outh1p,
        tc.tile_pool(name="smallp", bufs=2) as smallp,
        tc.tile_pool(name="ystp", bufs=3) as ystp,
        tc.tile_pool(name="dramp", bufs=4, space="DRAM") as dramp,
        tc.tile_pool(name="ps_work", bufs=2, space="PSUM") as ps_work,
        tc.tile_pool(name="ps_scores", bufs=1, space="PSUM") as ps_scores,
        tc.tile_pool(name="ps_av", bufs=2, space="PSUM") as ps_av,
    ):
        # ---- constants ----
        # mask_tri01[p, c] = 1 if c >= p else 0; multiplied into attnT after
        # the exp on diagonal blocks
        mask_tri01 = consts.tile([128, 128], MM_DT)
        nc.gpsimd.memset(mask_tri01, 1.0)
        nc.gpsimd.affine_select(
            out=mask_tri01, in_=mask_tri01,
            compare_op=mybir.AluOpType.is_ge,
            fill=0.0, base=0, pattern=[[1, 128]], channel_multiplier=-1,
        )

        # weights: [D, E_LOC] -> [128p, N_D, E_LOC] (contraction on partitions)
        # spread the startup DMAs over several engine queues so the loads
        # run in parallel and the PE isn't starved (and HAM-throttled) early
        w_sb = {}
        for (name, w), eng in zip(
            (("v", wvT), ("q", wqT), ("k", wkT)),
            (nc.gpsimd, nc.sync, nc.scalar),
        ):
            t = consts.tile([128, N_D, E_LOC], MM_DT, tag=f"w{name}_sb")
            eng.dma_start(out=t, in_=w[:].rearrange("(t p) e -> p t e", p=128))
            w_sb[name] = t
        ones32 = consts.tile([128, 128], FP32)
        nc.gpsimd.memset(ones32, 1.0)

        # ---- PE warmup ----
        # the HAM clock gate starts the PE at 1.2 GHz and only releases to
        # 2.4 GHz after ~3.4us of sustained activity; spin small matmuls
        # during the X DMA so the real work starts warm
        warm128 = consts.tile([128, 128], MM_DT)
        nc.gpsimd.memset(warm128, 0.0)
        warm_ps = ps_work.tile([128, SQ], FP32, tag="ps_work")
        for _ in range(128):
            mm(
                warm_ps[:, 0:128], lhsT=warm128, rhs=warm128,
                start=True, stop=True,
            )

        # ---- X^T (one batch), loaded once ----
        xt_sb = [
            consts.tile([128, S], MM_DT, tag=f"xt{t}", name=f"xt_sb{t}")
            for t in range(N_D)
        ]
        xt_engs = (nc.sync, nc.scalar, nc.gpsimd)
        for t in range(N_D):
            xt_engs[t % 3].dma_start(
                out=xt_sb[t], in_=xt[128 * t:128 * (t + 1), :]
            )
        # woutT [E_LOC, D] -> [128p, N_HP, D]; k-tile t == head-pair t.
        # Triggered last: it is only read by the output projection, and the
        # startup window is HBM-bandwidth-bound
        wout_sb = consts.tile([128, N_HP, D], MM_DT)
        nc.scalar.dma_start(
            out=wout_sb, in_=woutT[:].rearrange("(t p) e -> p t e", p=128)
        )

        # ---- V in natural layout, all 8 heads, with ones columns ----
        # v_sb[:, i, hp, :] = [V_h0(64) | 1 | V_h1(64) | 1] for kpos tile i
        v_sb = consts.tile([128, N_SK, N_HP, VBLK], MM_DT)
        ones_ap = bass.AP(
            tensor=v_sb.tensor,
            offset=v_sb.offset + DV,
            ap=[v_sb.ap[0], [VBLK, N_SK * N_HP], [DV + 1, 2]],
        )
        nc.vector.tensor_copy(
            ones_ap,
            bass.AP(
                tensor=ones32.tensor,
                offset=ones32.offset,
                ap=[ones32.ap[0], [2, N_SK * N_HP], [1, 2]],
            ),
        )
        def vnat_mms():
            """Yield per-matmul closures computing natural-layout V tiles."""
            for i in range(N_SK):
                vps = ps_work.tile([128, E_LOC], FP32, tag="ps_work")

                def emit(d, i=i, vps=vps):
                    mm(
                        vps,
                        lhsT=xt_sb[d][:, bass.ts(i, SK)],
                        rhs=w_sb["v"][:, d, :],
                        start=(d == 0),
                        stop=(d == N_D - 1),
                    )
                    if d != N_D - 1:
                        return
                    # scatter [128, hp*128 + 64h + c] -> v_sb[:, i, hp, 65h+c]
                    for half in range(2):
                        vdst = bass.AP(
                            tensor=v_sb.tensor,
                            offset=v_sb.offset + i * (N_HP * VBLK) + half * (DV + 1),
                            ap=[v_sb.ap[0], [VBLK, N_HP], [1, DV]],
                        )
                        vsrc = bass.AP(
                            tensor=vps.tensor,
                            offset=vps.offset + half * DV,
                            ap=[vps.ap[0], [128, N_HP], [1, DV]],
                        )
                        nc.vector.tensor_copy(vdst, vsrc)

                for d in range(N_D):
                    yield emit, d

        # ---- Q/K projection emitter (consumed inline or as fillers) ----
        qk_tiles = {}

        QK_DEFAULT = [(w, j) for j in range(N_SQ) for w in ("q", "k")]
        # hp3's q/k chunks for the late j-blocks are not needed until well
        # into the phase (block j starts at pair 2j^2+2j), so they are held
        # back as fillers for the thin early blocks of attn3 itself
        QK3_PRIMARY = [(w, j) for j in range(2) for w in ("q", "k")]
        QK3_RESERVE = [(w, j) for j in range(2, N_SQ) for w in ("k", "q")]

        def qk_mms(hp, chunks):
            if hp not in qk_tiles:
                qk_tiles[hp] = (
                    qkp.tile([128, S], MM_DT, tag="qT", name=f"qT{hp}"),
                    qkp.tile([128, S], MM_DT, tag="kT", name=f"kT{hp}"),
                )
            qT, kT = qk_tiles[hp]
            for wname, j in chunks:
                dst = qT if wname == "q" else kT
                ps = ps_work.tile([128, SQ], FP32, tag="ps_work")

                def emit(d, ps=ps, wname=wname, dst=dst, j=j):
                    mm(
                        ps,
                        lhsT=w_sb[wname][:, d, bass.ts(hp, 128)],
                        rhs=xt_sb[d][:, bass.ts(j, SQ)],
                        start=(d == 0),
                        stop=(d == N_D - 1),
                    )
                    if d == N_D - 1:
                        nc.vector.tensor_copy(dst[:, bass.ts(j, SQ)], ps)

                for d in range(N_D):
                    yield emit, d

        def hp0_prefix():
            """Interleaved V-nat + QK(hp0) stream with per-j barriers.

            Yields ('marker', j) sentinels; everything before marker j must be
            emitted before attn(hp0, j) starts: V tiles i <= 4j+3 and the
            j-th q/k projection chunks.
            """
            vgen = vnat_mms()
            qgen = qk_mms(0, QK_DEFAULT)
            for j in range(N_SQ):
                for _ in range(4 * N_D):       # V-nat tiles 4j..4j+3
                    yield next(vgen)
                for _ in range(2 * N_D):       # q/k chunk j
                    yield next(qgen)
                yield "marker", j

        # output projection tile emitter (used as fillers during hp3's attn)
        outT_all = [
            outp.tile([128, S], MM_DT, tag=f"outT{hp}", name=f"outT{hp}")
            for hp in range(N_HP)
        ]

        def yproj_tile(t):
            yst = ystp.tile([128, D], FP32)
            state = {}

            def emit(nk, yst=yst, t=t):
                n, hpk = divmod(nk, N_HP)
                if hpk == 0:
                    state[n] = ps_work.tile(
                        [128, SQ], FP32, tag="ps_work", name=f"yps{t}_{n}"
                    )
                yps = state[n]
                mm(
                    yps,
                    lhsT=outT_all[hpk][:, bass.ts(t, 128)],
                    rhs=wout_sb[:, hpk, bass.ts(n, SQ)],
                    start=(hpk == 0),
                    stop=(hpk == N_HP - 1),
                )
                if hpk == N_HP - 1:
                    # split the PSUM evacuation across both capable engines
                    if n == 0:
                        nc.scalar.copy(yst[:, 0:SQ], yps)
                    else:
                        nc.vector.tensor_copy(yst[:, SQ:D], yps)
                        nc.sync.dma_start(
                            out=y[128 * t:128 * (t + 1), :], in_=yst
                        )

            for nk in range(2 * N_HP):
                yield emit, nk

        # ---- unified filler queue ----
        # V-nat + QK0 (with per-j barriers for attn0), then QK1..QK3. Pulled
        # at a steady rate between attention pairs so the in-order PE queue
        # has projection work while the scalar engine computes exps.
        from collections import deque

        # base stream: V-nat + QK0, consumed via per-j barriers during attn0
        base_gen = hp0_prefix()
        seen_markers = set()

        def base_next():
            item = next(base_gen, None)
            if item is None:
                return False
            a, b = item
            if a == "marker":
                seen_markers.add(b)
            else:
                a(b)
            return True

        def ensure_base(j):
            while j not in seen_markers:
                if not base_next():
                    return

        # phase filler queue: refilled per phase, paced so it stretches to
        # the phase's last pair (ACT-bound stretches keep PE fed; no >3.4us
        # PE idle windows that would re-throttle the HAM clock gate)
        phase_q = deque()
        phase_count = 0
        pairs_left = 1
        filler_budget = 0.0

        def phase_load(gen, count):
            nonlocal phase_count
            phase_q.append(gen)
            phase_count += count

        def phase_drain():
            nonlocal phase_count
            while phase_q:
                item = next(phase_q[0], None)
                if item is None:
                    phase_q.popleft()
                    continue
                item[0](item[1])
                phase_count -= 1
            phase_count = 0

        def emit_fillers():
            nonlocal filler_budget, phase_count
            filler_budget += phase_count / max(1, pairs_left)
            while filler_budget >= 1.0 and phase_q:
                item = next(phase_q[0], None)
                if item is None:
                    phase_q.popleft()
                    continue
                item[0](item[1])
                phase_count -= 1
                filler_budget -= 1.0

        # scores PSUM and attnT SBUF: single buffers with two manually
        # rotated pair-slots (columns [0,2SQ) and [2SQ,4SQ)); consecutive
        # full-width pairs land in adjacent slots so ONE contiguous exp
        # activation covers both, halving the scalar engine's per-op
        # overhead on the hot path
        sc_big = ps_scores.tile([128, 4 * SQ], FP32, tag="sc_big")
        # two batch-slots deep so a batch's exp (WAR on the previous batch's
        # AV reads) never serializes against it
        at_big = attnp.tile([128, 8 * SQ], MM_DT)

        def abase(i):
            return 4 * SQ * ((i // 2) % 2) + 2 * SQ * (i % 2)

        # ---- attention, one head-pair at a time ----
        for hp in range(N_HP):
            if hp == 0:
                ensure_base(0)
            else:
                phase_drain()  # leftover QK(hp) fillers from previous phase
            if hp + 1 < N_HP - 1:
                phase_load(qk_mms(hp + 1, QK_DEFAULT), len(QK_DEFAULT) * N_D)
            elif hp + 1 == N_HP - 1:
                phase_load(qk_mms(hp + 1, QK3_PRIMARY), len(QK3_PRIMARY) * N_D)
            else:
                phase_load(qk_mms(hp, QK3_RESERVE), len(QK3_RESERVE) * N_D)
            pairs_left = sum(4 * j + 4 for j in range(N_SQ))

            qT_sb, kT_sb = qk_tiles[hp]

            scope_att = nc.named_scope(f"attn{hp}"); scope_att.__enter__()
            outT_full = outT_all[hp]
            outT_h1 = outh1p.tile([64, S], MM_DT, tag="outT_h1")
            for j in range(N_SQ):
                if hp == 0:
                    ensure_base(j)
                av_ps = [
                    ps_av.tile([DV + 1, SQ], FP32, tag="av_ps", name=f"av_ps{h}")
                    for h in range(2)
                ]
                n_i = 4 * j + 4
                if hp == N_HP - 1:
                    # pace fillers per j-block: the previous block's output-
                    # projection tiles must land inside this block, not
                    # stretch to phase end (early blocks are tiny)
                    pairs_left = n_i
                def post_exp(i, s0, hp=hp, j=j, n_i=n_i):
                    """Mask (diagonal), fillers, and AV matmuls for pair i."""
                    nonlocal pairs_left
                    base = abase(i)
                    if i >= 4 * j:  # diagonal: zero upper-triangle entries
                        blk = bass.AP(
                            tensor=at_big.tensor,
                            offset=at_big.offset + base + s0,
                            ap=[at_big.ap[0], [SQ, 2], [1, 128]],
                        )
                        mask2 = bass.AP(
                            tensor=mask_tri01.tensor,
                            offset=mask_tri01.offset,
                            ap=[mask_tri01.ap[0], [0, 2], [1, 128]],
                        )
                        nc.vector.tensor_mul(blk, blk, mask2)
                    emit_fillers()
                    pairs_left -= 1
                    for h in range(2):
                        mm(
                            av_ps[h][:, s0:SQ],
                            lhsT=v_sb[:, i, hp, h * (DV + 1):(h + 1) * (DV + 1)],
                            rhs=at_big[:, base + SQ * h + s0:base + SQ * (h + 1)],
                            start=(i == 0),
                            stop=(i == n_i - 1),
                        )

                pending = None
                for i in range(n_i):
                    s0 = max(0, SK * i - SQ * j)
                    w = SQ - s0
                    base = 2 * SQ * (i % 2)
                    for h in range(2):
                        mm(
                            sc_big[:, base + SQ * h + s0:base + SQ * (h + 1)],
                            lhsT=kT_sb[DV * h:DV * (h + 1), bass.ts(i, SK)],
                            rhs=qT_sb[DV * h:DV * (h + 1), SQ * j + s0:SQ * (j + 1)],
                            start=True,
                            stop=True,
                        )
                    if w == SQ and i % 2 == 0 and i + 1 <= 4 * j:
                        pending = i  # next pair is also full-width: batch
                        continue
                    if pending is not None:
                        # one exp over both pair-slots (contiguous 4*SQ)
                        bdst = bass.AP(
                            tensor=at_big.tensor,
                            offset=at_big.offset + 4 * SQ * ((i // 2) % 2),
                            ap=[at_big.ap[0], [1, 4 * SQ]],
                        )
                        nc.scalar.activation(
                            bdst, sc_big, mybir.ActivationFunctionType.Exp
                        )
                        post_exp(pending, 0)
                        pending = None
                    else:
                        src = bass.AP(
                            tensor=sc_big.tensor,
                            offset=sc_big.offset + base + s0,
                            ap=[sc_big.ap[0], [SQ, 2], [1, w]],
                        )
                        dst = bass.AP(
                            tensor=at_big.tensor,
                            offset=at_big.offset + abase(i) + s0,
                            ap=[at_big.ap[0], [SQ, 2], [1, w]],
                        )
                        nc.scalar.activation(
                            dst, src, mybir.ActivationFunctionType.Exp
                        )
                    post_exp(i, s0)
                # evacuate BOTH av psum banks first: the next j-block's AV
                # matmuls rotate onto these banks, and the normalize chain
                # below blocks the DVE FIFO on a DMA-broadcast round trip
                av_st = []
                for h in range(2):
                    st = smallp.tile(
                        [DV + 1, SQ], FP32, tag="av_st", name=f"av_st{h}"
                    )
                    nc.vector.tensor_copy(st, av_ps[h])
                    av_st.append(st)
                for h in range(2):
                    # stage the denom row at partition 0: the custom-DVE
                    # approx reciprocal mangles partition-offset inputs
                    den = smallp.tile([1, SQ], FP32, tag="den")
                    nc.vector.tensor_copy(den, av_st[h][DV:DV + 1, :])
                    recip = smallp.tile([1, SQ], FP32, tag="recip")
                    nc.vector.reciprocal_approx_fast(recip, den)
                    bcast = smallp.tile([DV, SQ], FP32, tag="bcast")
                    nc.gpsimd.partition_broadcast(bcast, recip)
                    dst = (
                        outT_full[0:DV, bass.ts(j, SQ)]
                        if h == 0
                        else outT_h1[:, bass.ts(j, SQ)]
                    )
                    nc.vector.tensor_mul(dst, av_st[h][0:DV, :], bcast)
                # during hp3's attention, trail the output projection one
                # j-block behind (its deps for j-1 are complete for all hps)
                if hp == N_HP - 1 and j > 0:
                    # block j-1 is complete for ALL head-pairs (with a full
                    # j-block of settling margin): shift its head1 rows and
                    # feed its output-projection tiles in as fillers
                    nc.gpsimd.dma_start(
                        out=outT_full[DV:128, bass.ts(j - 1, SQ)],
                        in_=outT_h1[:, bass.ts(j - 1, SQ)],
                    )
                    for t in range(4 * (j - 1), 4 * j):
                        phase_load(yproj_tile(t), 2 * N_HP)
            # shift head1 rows to partitions 64..127
            if hp == N_HP - 1:
                nc.gpsimd.dma_start(
                    out=outT_full[DV:128, bass.ts(N_SQ - 1, SQ)],
                    in_=outT_h1[:, bass.ts(N_SQ - 1, SQ)],
                )
            else:
                nc.gpsimd.dma_start(out=outT_full[DV:128, :], in_=outT_h1)
            scope_att.__exit__(None, None, None)

        # ---- output projection tail (last j-block) ----
        scope_y = nc.named_scope("yproj"); scope_y.__enter__()
        phase_drain()
        for t in range(4 * (N_SQ - 1), 4 * N_SQ):
            phase_load(yproj_tile(t), 2 * N_HP)
        phase_drain()
        scope_y.__exit__(None, None, None)


def shard_inputs(X, W_qkv, W_out):
    """Host-side sharding. Core c = (batch c//2, head-group c%2)."""
    X = np.asarray(X, dtype=np.float32)
    W_qkv = np.asarray(W_qkv, dtype=np.float32)
    W_out = np.asarray(W_out, dtype=np.float32)
    np_mm = mybir.dt.np(MM_DT)
    xt = np.ascontiguousarray(X.transpose(0, 2, 1)).astype(np_mm)  # [B, D, S]
    scale = np.float32(1.0 / np.sqrt(DV))
    in_maps = []
    for c in range(N_CORES):
        b, g = divmod(c, 2)
        r = slice(E_LOC * g, E_LOC * (g + 1))
        wq = W_qkv[0 * D:1 * D][r] * scale
        wk = W_qkv[1 * D:2 * D][r]
        wv = W_qkv[2 * D:3 * D][r]
        in_maps.append({
            "xt": xt[b],
            "wqT": np.ascontiguousarray(wq.T).astype(np_mm),
            "wkT": np.ascontiguousarray(wk.T).astype(np_mm),
            "wvT": np.ascontiguousarray(wv.T).astype(np_mm),
            "woutT": np.ascontiguousarray(W_out[:, r].T).astype(np_mm),
        })
    return in_maps


def kernel(X, W_qkv, W_out):
    from concourse.bass_utils import run_bass_kernel_spmd

    nc = build_nc()
    in_maps = shard_inputs(X, W_qkv, W_out)
    res = run_bass_kernel_spmd(nc, in_maps, core_ids=list(range(N_CORES)))
    out = np.zeros((B, S, D), dtype=np.float32)
    for c, r in enumerate(res.results):
        out[c // 2] += r["y"]
    return out


# revision 65
# speedup vs baseline: 1.2053x; 1.2053x over previous
"""Causal multi-head self-attention on 8 Trainium2 NeuronCores.

Sharding: hybrid (batch x head-half) tensor parallel. B=4 batches x 2
head-groups of 8 heads = 8 shards. Core c = (b = c//2, g = c%2) gets batch
b's X (transposed, loaded once) and the W_qkv rows / W_out columns for heads
8g..8g+7, computes those heads' attention plus its half of the output
projection contraction, and returns a partial [S, D] output. The host sums
the 2 partials per batch (the "all-reduce" of the TP output projection).

vs the pure head-parallel V1: X DMA per core drops 16MB -> 4MB (loaded
once), y output 32MB -> 8MB, and the output projection accumulates its 4
contraction tiles in PSUM so it is evacuated once instead of 4x.

On-device layout (per core; everything transposed until the end):
  - v natural [kpos, e] is computed directly by matmul (lhsT = X^T tile,
    rhs = W_v^T tile) -- no PE transposes -- and scattered into per-head-pair
    blocks [V_h0 | 1 | V_h1 | 1]; the ones columns make the attn@V matmul
    emit softmax denominators in PSUM row 64 for free.
  - qT/kT per head-pair: [128 rows = 2 heads x dv, S]; head0 on partitions
    0-63, head1 on 64-127.
  - scoresT[kpos, q] = matmul(lhsT=kT tile, rhs=qT tile) per head; K=64 with
    lhsT/rhs base partitions 0/64 auto-derives PE row-tiling positions
    (0,0)/(64,0), so the two heads' score matmuls run concurrently on the
    array halves.
  - causal: only q >= kpos tiles/columns computed (partial-width matmuls);
    the diagonal 128-block gets a 0/1 triangular mask multiplied after exp.
  - softmax without max-subtraction (scores ~ N(0,1); fp32 exp is safe);
    exp on the scalar engine reads PSUM directly, one call per pair covering
    both heads.
  - normalization: reciprocal of the PSUM denominator row, partition-
    broadcast via a DRAM bounce, one DVE multiply into outT.
  - emission interleaving: the next head-pair's Q/K projection matmuls are
    emitted between attention pairs ("fillers") so the in-order PE queue has
    work while the scalar engine computes exp; the output projection tiles
    are likewise interleaved into the last head-pair's attention.
"""

import numpy as np

import concourse.bacc as bacc
import concourse.bass as bass
import concourse.mybir as mybir
import concourse.tile as tile

FP32 = mybir.dt.float32

B = 4
S = 2048
D = 1024
H = 16
DV = 64
N_CORES = 8
H_LOC = 8                  # heads per core
N_HP = H_LOC // 2          # head-pairs per core
E_LOC = H_LOC * DV         # 512 rows of Q/K/V per core
MM_DT = mybir.dt.float16

SQ = 512            # q tile width (PSUM bank)
SK = 128            # kpos tile width
N_SQ = S // SQ      # 4 q-tiles
N_SK = S // SK      # 16 kpos tiles
N_D = D // 128      # 8 contraction tiles for QKV
VBLK = 2 * (DV + 1)  # 130: [V_h0 | 1 | V_h1 | 1] per (kpos tile, head-pair)


def build_nc() -> bass.Bass:
    # Bacc (not plain Bass): its compile() pass splits multi-wait
    # instructions that walrus codegen otherwise rejects.
    nc = bacc.Bacc(None, target_bir_lowering=False)

    xt = nc.declare_dram_parameter("xt", [D, S], MM_DT, isOutput=False)
    wqT = nc.declare_dram_parameter("wqT", [D, E_LOC], MM_DT, isOutput=False)
    wkT = nc.declare_dram_parameter("wkT", [D, E_LOC], MM_DT, isOutput=False)
    wvT = nc.declare_dram_parameter("wvT", [D, E_LOC], MM_DT, isOutput=False)
    woutT = nc.declare_dram_parameter("woutT", [E_LOC, D], MM_DT, isOutput=False)
    y = nc.declare_dram_parameter("y", [S, D], FP32, isOutput=True)

    with tile.TileContext(nc) as tc:
        _build(tc, xt, wqT, wkT, wvT, woutT, y)
    nc.compile()
    return nc


def _build(tc, xt, wqT, wkT, wvT, woutT, y):
    nc = tc.nc

    def mm(out, lhsT, rhs, start, stop):
        nc.tensor.matmul(out, lhsT=lhsT, rhs=rhs, start=start, stop=stop)

    with (
        tc.tile_pool(name="consts", bufs=1) as consts,
        tc.tile_pool(name="qkp", bufs=2) as qkp,
        tc.tile_pool(name="attnp", bufs=3) as attnp,
        tc.tile_pool(name="outp", bufs=1) as outp,
        tc.tile_pool(name="outh1p", bufs=2) as outh1p,
        tc.tile_pool(name="smallp", bufs=2) as smallp,
        tc.tile_pool(name="ystp", bufs=3) as ystp,
        tc.tile_pool(name="dramp", bufs=4, space="DRAM") as dramp,
        tc.tile_pool(name="ps_work", bufs=2, space="PSUM") as ps_work,
        tc.tile_pool(name="ps_scores", bufs=2, space="PSUM") as ps_scores,
        tc.tile_pool(name="ps_av", bufs=2, space="PSUM") as ps_av,
    ):
        # ---- constants ----
        # mask_tri01[p, c] = 1 if c >= p else 0; multiplied into attnT after
        # the exp on diagonal blocks
        mask_tri01 = consts.tile([128, 128], MM_DT)
        nc.gpsimd.memset(mask_tri01, 1.0)
        nc.gpsimd.affine_select(
            out=mask_tri01, in_=mask_tri01,
            compare_op=mybir.AluOpType.is_ge,
            fill=0.0, base=0, pattern=[[1, 128]], channel_multiplier=-1,
        )

        # weights: [D, E_LOC] -> [128p, N_D, E_LOC] (contraction on partitions)
        # spread the startup DMAs over several engine queues so the loads
        # run in parallel and the PE isn't starved (and HAM-throttled) early
        w_sb = {}
        for (name, w), eng in zip(
            (("v", wvT), ("q", wqT), ("k", wkT)),
            (nc.gpsimd, nc.sync, nc.scalar),
        ):
            t = consts.tile([128, N_D, E_LOC], MM_DT, tag=f"w{name}_sb")
            eng.dma_start(out=t, in_=w[:].rearrange("(t p) e -> p t e", p=128))
            w_sb[name] = t
        ones32 = consts.tile([128, 128], FP32)
        nc.gpsimd.memset(ones32, 1.0)

        # ---- PE warmup ----
        # the HAM clock gate starts the PE at 1.2 GHz and only releases to
        # 2.4 GHz after ~3.4us of sustained activity; spin small matmuls
        # during the X DMA so the real work starts warm
        warm128 = consts.tile([128, 128], MM_DT)
        nc.gpsimd.memset(warm128, 0.0)
        warm_ps = ps_work.tile([128, SQ], FP32, tag="ps_work")
        for _ in range(128):
            mm(
                warm_ps[:, 0:128], lhsT=warm128, rhs=warm128,
                start=True, stop=True,
            )

        # ---- X^T (one batch), loaded once ----
        xt_sb = [
            consts.tile([128, S], MM_DT, tag=f"xt{t}", name=f"xt_sb{t}")
            for t in range(N_D)
        ]
        xt_engs = (nc.sync, nc.scalar, nc.gpsimd)
        for t in range(N_D):
            xt_engs[t % 3].dma_start(
                out=xt_sb[t], in_=xt[128 * t:128 * (t + 1), :]
            )
        # woutT [E_LOC, D] -> [128p, N_HP, D]; k-tile t == head-pair t.
        # Triggered last: it is only read by the output projection, and the
        # startup window is HBM-bandwidth-bound
        wout_sb = consts.tile([128, N_HP, D], MM_DT)
        nc.scalar.dma_start(
            out=wout_sb, in_=woutT[:].rearrange("(t p) e -> p t e", p=128)
        )

        # ---- V in natural layout, all 8 heads, with ones columns ----
        # v_sb[:, i, hp, :] = [V_h0(64) | 1 | V_h1(64) | 1] for kpos tile i
        v_sb = consts.tile([128, N_SK, N_HP, VBLK], MM_DT)
        ones_ap = bass.AP(
            tensor=v_sb.tensor,
            offset=v_sb.offset + DV,
            ap=[v_sb.ap[0], [VBLK, N_SK * N_HP], [DV + 1, 2]],
        )
        nc.vector.tensor_copy(
            ones_ap,
            bass.AP(
                tensor=ones32.tensor,
                offset=ones32.offset,
                ap=[ones32.ap[0], [2, N_SK * N_HP], [1, 2]],
            ),
        )
        def vnat_mms():
            """Yield per-matmul closures computing natural-layout V tiles."""
            for i in range(N_SK):
                vps = ps_work.tile([128, E_LOC], FP32, tag="ps_work")

                def emit(d, i=i, vps=vps):
                    mm(
                        vps,
                        lhsT=xt_sb[d][:, bass.ts(i, SK)],
                        rhs=w_sb["v"][:, d, :],
                        start=(d == 0),
                        stop=(d == N_D - 1),
                    )
                    if d != N_D - 1:
                        return
                    # scatter [128, hp*128 + 64h + c] -> v_sb[:, i, hp, 65h+c]
                    for half in range(2):
                        vdst = bass.AP(
                            tensor=v_sb.tensor,
                            offset=v_sb.offset + i * (N_HP * VBLK) + half * (DV + 1),
                            ap=[v_sb.ap[0], [VBLK, N_HP], [1, DV]],
                        )
                        vsrc = bass.AP(
                            tensor=vps.tensor,
                            offset=vps.offset + half * DV,
                            ap=[vps.ap[0], [128, N_HP], [1, DV]],
                        )
                        nc.vector.tensor_copy(vdst, vsrc)

                for d in range(N_D):
                    yield emit, d

        # ---- Q/K projection emitter (consumed inline or as fillers) ----
        qk_tiles = {}

        QK_DEFAULT = [(w, j) for j in range(N_SQ) for w in ("q", "k")]
        # hp3's q/k chunks for the late j-blocks are not needed until well
        # into the phase (block j starts at pair 2j^2+2j), so they are held
        # back as fillers for the thin early blocks of attn3 itself
        QK3_PRIMARY = [(w, j) for j in range(2) for w in ("q", "k")]
        QK3_RESERVE = [(w, j) for j in range(2, N_SQ) for w in ("k", "q")]

        def qk_mms(hp, chunks):
            if hp not in qk_tiles:
                qk_tiles[hp] = (
                    qkp.tile([128, S], MM_DT, tag="qT", name=f"qT{hp}"),
                    qkp.tile([128, S], MM_DT, tag="kT", name=f"kT{hp}"),
                )
            qT, kT = qk_tiles[hp]
            for wname, j in chunks:
                dst = qT if wname == "q" else kT
                ps = ps_work.tile([128, SQ], FP32, tag="ps_work")

                def emit(d, ps=ps, wname=wname, dst=dst, j=j):
                    mm(
                        ps,
                        lhsT=w_sb[wname][:, d, bass.ts(hp, 128)],
                        rhs=xt_sb[d][:, bass.ts(j, SQ)],
                        start=(d == 0),
                        stop=(d == N_D - 1),
                    )
                    if d == N_D - 1:
                        nc.vector.tensor_copy(dst[:, bass.ts(j, SQ)], ps)

                for d in range(N_D):
                    yield emit, d

        def hp0_prefix():
            """Interleaved V-nat + QK(hp0) stream with per-j barriers.

            Yields ('marker', j) sentinels; everything before marker j must be
            emitted before attn(hp0, j) starts: V tiles i <= 4j+3 and the
            j-th q/k projection chunks.
            """
            vgen = vnat_mms()
            qgen = qk_mms(0, QK_DEFAULT)
            for j in range(N_SQ):
                for _ in range(4 * N_D):       # V-nat tiles 4j..4j+3
                    yield next(vgen)
                for _ in range(2 * N_D):       # q/k chunk j
                    yield next(qgen)
                yield "marker", j

        # output projection tile emitter (used as fillers during hp3's attn)
        outT_all = [
            outp.tile([128, S], MM_DT, tag=f"outT{hp}", name=f"outT{hp}")
            for hp in range(N_HP)
        ]

        def yproj_tile(t):
            yps = ps_scores.tile([128, D], FP32, tag="sc_ps")

            def emit(nk, yps=yps, t=t):
                n, hpk = divmod(nk, N_HP)
                mm(
                    yps[:, bass.ts(n, SQ)],
                    lhsT=outT_all[hpk][:, bass.ts(t, 128)],
                    rhs=wout_sb[:, hpk, bass.ts(n, SQ)],
                    start=(hpk == 0),
                    stop=(hpk == N_HP - 1),
                )
                if nk == 2 * N_HP - 1:
                    yst = ystp.tile([128, D], FP32)
                    # split the PSUM evacuation across both capable engines
                    nc.scalar.copy(yst[:, 0:SQ], yps[:, 0:SQ])
                    nc.vector.tensor_copy(yst[:, SQ:D], yps[:, SQ:D])
                    nc.sync.dma_start(out=y[128 * t:128 * (t + 1), :], in_=yst)

            for nk in range(2 * N_HP):
                yield emit, nk

        # ---- unified filler queue ----
        # V-nat + QK0 (with per-j barriers for attn0), then QK1..QK3. Pulled
        # at a steady rate between attention pairs so the in-order PE queue
        # has projection work while the scalar engine computes exps.
        from collections import deque

        # base stream: V-nat + QK0, consumed via per-j barriers during attn0
        base = hp0_prefix()
        seen_markers = set()

        def base_next():
            item = next(base, None)
            if item is None:
                return False
            a, b = item
            if a == "marker":
                seen_markers.add(b)
            else:
                a(b)
            return True

        def ensure_base(j):
            while j not in seen_markers:
                if not base_next():
                    return

        # phase filler queue: refilled per phase, paced so it stretches to
        # the phase's last pair (ACT-bound stretches keep PE fed; no >3.4us
        # PE idle windows that would re-throttle the HAM clock gate)
        phase_q = deque()
        phase_count = 0
        pairs_left = 1
        filler_budget = 0.0

        def phase_load(gen, count):
            nonlocal phase_count
            phase_q.append(gen)
            phase_count += count

        def phase_drain():
            nonlocal phase_count
            while phase_q:
                item = next(phase_q[0], None)
                if item is None:
                    phase_q.popleft()
                    continue
                item[0](item[1])
                phase_count -= 1
            phase_count = 0

        def emit_fillers():
            nonlocal filler_budget, phase_count
            filler_budget += phase_count / max(1, pairs_left)
            while filler_budget >= 1.0 and phase_q:
                item = next(phase_q[0], None)
                if item is None:
                    phase_q.popleft()
                    continue
                item[0](item[1])
                phase_count -= 1
                filler_budget -= 1.0

        # ---- attention, one head-pair at a time ----
        for hp in range(N_HP):
            if hp == 0:
                ensure_base(0)
            else:
                phase_drain()  # leftover QK(hp) fillers from previous phase
            if hp + 1 < N_HP - 1:
                phase_load(qk_mms(hp + 1, QK_DEFAULT), len(QK_DEFAULT) * N_D)
            elif hp + 1 == N_HP - 1:
                phase_load(qk_mms(hp + 1, QK3_PRIMARY), len(QK3_PRIMARY) * N_D)
            else:
                phase_load(qk_mms(hp, QK3_RESERVE), len(QK3_RESERVE) * N_D)
            pairs_left = sum(4 * j + 4 for j in range(N_SQ))

            qT_sb, kT_sb = qk_tiles[hp]

            scope_att = nc.named_scope(f"attn{hp}"); scope_att.__enter__()
            outT_full = outT_all[hp]
            outT_h1 = outh1p.tile([64, S], MM_DT, tag="outT_h1")
            for j in range(N_SQ):
                if hp == 0:
                    ensure_base(j)
                av_ps = [
                    ps_av.tile([DV + 1, SQ], FP32, tag="av_ps", name=f"av_ps{h}")
                    for h in range(2)
                ]
                n_i = 4 * j + 4
                if hp == N_HP - 1:
                    # pace fillers per j-block: the previous block's output-
                    # projection tiles must land inside this block, not
                    # stretch to phase end (early blocks are tiny)
                    pairs_left = n_i
                for i in range(n_i):
                    s0 = max(0, SK * i - SQ * j)
                    w = SQ - s0
                    sc_ps = ps_scores.tile([128, 2 * SQ], FP32, tag="sc_ps")
                    for h in range(2):
                        mm(
                            sc_ps[:, SQ * h + s0:SQ * (h + 1)],
                            lhsT=kT_sb[DV * h:DV * (h + 1), bass.ts(i, SK)],
                            rhs=qT_sb[DV * h:DV * (h + 1), SQ * j + s0:SQ * (j + 1)],
                            start=True,
                            stop=True,
                        )
                    attnT = attnp.tile([128, 2 * SQ], MM_DT)
                    src = bass.AP(
                        tensor=sc_ps.tensor,
                        offset=sc_ps.offset + s0,
                        ap=[sc_ps.ap[0], [SQ, 2], [1, w]],
                    )
                    dst = bass.AP(
                        tensor=attnT.tensor,
                        offset=attnT.offset + s0,
                        ap=[attnT.ap[0], [SQ, 2], [1, w]],
                    )
                    nc.scalar.activation(dst, src, mybir.ActivationFunctionType.Exp)
                    if i >= 4 * j:  # diagonal: zero upper-triangle entries
                        blk = bass.AP(
                            tensor=attnT.tensor,
                            offset=attnT.offset + s0,
                            ap=[attnT.ap[0], [SQ, 2], [1, 128]],
                        )
                        mask2 = bass.AP(
                            tensor=mask_tri01.tensor,
                            offset=mask_tri01.offset,
                            ap=[mask_tri01.ap[0], [0, 2], [1, 128]],
                        )
                        nc.vector.tensor_mul(blk, blk, mask2)
                    emit_fillers()
                    pairs_left -= 1
                    for h in range(2):
                        mm(
                            av_ps[h][:, s0:SQ],
                            lhsT=v_sb[:, i, hp, h * (DV + 1):(h + 1) * (DV + 1)],
                            rhs=attnT[:, SQ * h + s0:SQ * (h + 1)],
                            start=(i == 0),
                            stop=(i == n_i - 1),
                        )
                # evacuate BOTH av psum banks first: the next j-block's AV
                # matmuls rotate onto these banks, and the normalize chain
                # below blocks the DVE FIFO on a DMA-broadcast round trip
                av_st = []
                for h in range(2):
                    st = smallp.tile(
                        [DV + 1, SQ], FP32, tag="av_st", name=f"av_st{h}"
                    )
                    nc.vector.tensor_copy(st, av_ps[h])
                    av_st.append(st)
                for h in range(2):
                    # stage the denom row at partition 0: the custom-DVE
                    # approx reciprocal mangles partition-offset inputs
                    den = smallp.tile([1, SQ], FP32, tag="den")
                    nc.vector.tensor_copy(den, av_st[h][DV:DV + 1, :])
                    recip = smallp.tile([1, SQ], FP32, tag="recip")
                    nc.vector.reciprocal_approx_fast(recip, den)
                    rbounce = dramp.tile([SQ], FP32, tag="rbounce")
                    nc.sync.dma_start(out=rbounce, in_=recip)
                    bcast = smallp.tile([DV, SQ], FP32, tag="bcast")
                    nc.gpsimd.dma_start(
                        out=bcast,
                        in_=bass.AP(
                            tensor=rbounce.tensor,
                            offset=rbounce.offset,
                            ap=[[0, DV], [1, SQ]],
                        ),
                    )
                    dst = (
                        outT_full[0:DV, bass.ts(j, SQ)]
                        if h == 0
                        else outT_h1[:, bass.ts(j, SQ)]
                    )
                    nc.vector.tensor_mul(dst, av_st[h][0:DV, :], bcast)
                # during hp3's attention, trail the output projection one
                # j-block behind (its deps for j-1 are complete for all hps)
                if hp == N_HP - 1 and j > 0:
                    # block j-1 is complete for ALL head-pairs (with a full
                    # j-block of settling margin): shift its head1 rows and
                    # feed its output-projection tiles in as fillers
                    nc.gpsimd.dma_start(
                        out=outT_full[DV:128, bass.ts(j - 1, SQ)],
                        in_=outT_h1[:, bass.ts(j - 1, SQ)],
                    )
                    for t in range(4 * (j - 1), 4 * j):
                        phase_load(yproj_tile(t), 2 * N_HP)
            # shift head1 rows to partitions 64..127
            if hp == N_HP - 1:
                nc.gpsimd.dma_start(
                    out=outT_full[DV:128, bass.ts(N_SQ - 1, SQ)],
                    in_=outT_h1[:, bass.ts(N_SQ - 1, SQ)],
                )
            else:
                nc.gpsimd.dma_start(out=outT_full[DV:128, :], in_=outT_h1)
            scope_att.__exit__(None, None, None)

        # ---- output projection tail (last j-block) ----
        scope_y = nc.named_scope("yproj"); scope_y.__enter__()
        phase_drain()
        for t in range(4 * (N_SQ - 1), 4 * N_SQ):
            phase_load(yproj_tile(t), 2 * N_HP)
        phase_drain()
        scope_y.__exit__(None, None, None)


def shard_inputs(X, W_qkv, W_out):
    """Host-side sharding. Core c = (batch c//2, head-group c%2)."""
    X = np.asarray(X, dtype=np.float32)
    W_qkv = np.asarray(W_qkv, dtype=np.float32)
    W_out = np.asarray(W_out, dtype=np.float32)
    np_mm = mybir.dt.np(MM_DT)
    xt = np.ascontiguousarray(X.transpose(0, 2, 1)).astype(np_mm)  # [B, D, S]
    scale = np.float32(1.0 / np.sqrt(DV))
    in_maps = []
    for c in range(N_CORES):
        b, g = divmod(c, 2)
        r = slice(E_LOC * g, E_LOC * (g + 1))
        wq = W_qkv[0 * D:1 * D][r] * scale
        wk = W_qkv[1 * D:2 * D][r]
        wv = W_qkv[2 * D:3 * D][r]
        in_maps.append({
            "xt": xt[b],
            "wqT": np.ascontiguousarray(wq.T).astype(np_mm),
            "wkT": np.ascontiguousarray(wk.T).astype(np_mm),
            "wvT": np.ascontiguousarray(wv.T).astype(np_mm),
            "woutT": np.ascontiguousarray(W_out[:, r].T).astype(np_mm),
        })
    return in_maps


def kernel(X, W_qkv, W_out):
    from concourse.bass_utils import run_bass_kernel_spmd

    nc = build_nc()
    in_maps = shard_inputs(X, W_qkv, W_out)
    res = run_bass_kernel_spmd(nc, in_maps, core_ids=list(range(N_CORES)))
    out = np.zeros((B, S, D), dtype=np.float32)
    for c, r in enumerate(res.results):
        out[c // 2] += r["y"]
    return out
